# revision 1
# baseline (speedup 1.0000x reference)
"""MoE expert-FFN kernel for Trainium2, expert-parallel across 8 NeuronCores.

Problem: out[t] = silu(x[t] @ W1[e_t]^T) @ W2[e_t]^T with
  E=64 experts, D=512, H=1024, T=256 tokens.

Strategy (memory-bound on expert weights, ~268MB fp32 total):
  - Core c owns experts [8c, 8c+8). Host routes tokens to the core owning
    their expert (the hint's all-to-all done on host since we hold full
    inputs), padding each expert's tokens to a fixed capacity C.
  - Host pre-packs weights into the exact SBUF layout so the device does
    nothing but stream 4MiB/expert with perfect 128-partition DMAs.
  - On device, weights are the MOVING matmul operand (N=512 columns,
    full-rate float32r) and the tiny token blocks are the stationary
    operand, so the PE streams each weight element exactly once:
       H = silu(W1T-tiles streamed against x^T)     [tok, 1024] in PSUM
       H^T via 8 PE-transposes                      [128, tok] chunks
       Y = W2T-tiles streamed against H^T           [tok, 512]
  - float32r: full 4-byte weights in HBM (memory regime unchanged) with
    single-pass PE streaming; ~1.8e-4 absmax-relative vs the fp32 oracle.
"""

import numpy as np

E, D, H, T = 64, 512, 1024, 256
NCORES = 8
EPC = E // NCORES          # experts per core
DC = D // 128              # 4 d-chunks
HC = H // 128              # 8 h-chunks
WCOLS = DC * H + HC * D    # 8192 free columns of packed weights per expert
CB = 32                    # token block (PE-transpose granularity)

_prog_cache = {}


def _build_program(C, w_bufs=6, wdt_name="f32r"):
    import concourse.mybir as mybir
    import concourse.tile as tile
    from concourse import bacc

    f32 = mybir.dt.float32
    wdt = {"f32": f32, "f32r": mybir.dt.float32r,
           "bf16": mybir.dt.bfloat16, "f16": mybir.dt.float16}[wdt_name]
    blocks = C // CB
    nc = bacc.Bacc("TRN2", target_bir_lowering=False, debug=False)

    wts = nc.dram_tensor("wts", [EPC, 128, WCOLS], wdt, kind="ExternalInput")
    xt = nc.dram_tensor("xt", [128, EPC * DC * C], wdt, kind="ExternalInput")
    idt = nc.dram_tensor("idt", [CB, CB], wdt, kind="ExternalInput")
    yt = nc.dram_tensor("yt", [EPC, blocks, CB, D], f32, kind="ExternalOutput")

    with tile.TileContext(nc) as tc:
        with (
            tc.tile_pool(name="wpool", bufs=w_bufs) as wpool,
            tc.tile_pool(name="xpool", bufs=1) as xpool,
            tc.tile_pool(name="cpool", bufs=1) as cpool,
            tc.tile_pool(name="hpool", bufs=2) as hpool,
            tc.tile_pool(name="ypool", bufs=2) as ypool,
            tc.tile_pool(name="psh", bufs=2, space="PSUM") as pshp,
            tc.tile_pool(name="pst", bufs=2, space="PSUM") as pstp,
            tc.tile_pool(name="psy", bufs=2, space="PSUM") as psyp,
        ):
            ident = cpool.tile([CB, CB], wdt)
            nc.sync.dma_start(ident[:], idt[:])
            ident_w = ident[:]
            xall = xpool.tile([128, EPC * DC * C], wdt)
            nc.sync.dma_start(xall[:], xt[:])

            for s in range(EPC):
                w1 = wpool.tile([128, DC * H], wdt, tag="w")
                nc.sync.dma_start(w1[:], wts[s][:, :DC * H])
                w2 = wpool.tile([128, HC * D], wdt, tag="w")
                nc.sync.dma_start(w2[:], wts[s][:, DC * H:])

                for b in range(blocks):
                    # ---- fc1: Hpre[t, h] = sum_d x^T[d, t] * W1T[d, h]
                    psh = pshp.tile([CB, H], f32, tag="psh")
                    for nh in range(2):
                        for c in range(DC):
                            nc.tensor.matmul(
                                psh[:, nh * 512:(nh + 1) * 512],
                                xall[:, (s * DC + c) * C + b * CB:
                                     (s * DC + c) * C + (b + 1) * CB],
                                w1[:, c * H + nh * 512: c * H + (nh + 1) * 512],
                                start=(c == 0),
                                stop=(c == DC - 1),
                            )

                    # ---- silu: h = psh * sigmoid(psh)   [CB, 1024] -> SBUF
                    sig = hpool.tile([CB, H], f32, tag="sig")
                    nc.scalar.activation(
                        sig[:], psh[:], mybir.ActivationFunctionType.Sigmoid
                    )
                    hbuf = hpool.tile([CB, H], wdt, tag="h")
                    nc.vector.tensor_mul(hbuf[:], psh[:], sig[:])

                    # ---- transpose h -> hT [128, HC*CB] via PE
                    pst = pstp.tile([128, HC * CB], wdt, tag="pst")
                    for ch in range(HC):
                        nc.tensor.transpose(
                            pst[:, ch * CB:(ch + 1) * CB],
                            hbuf[:, ch * 128:(ch + 1) * 128],
                            ident_w,
                        )
                    ht = hpool.tile([128, HC * CB], wdt, tag="ht")
                    nc.vector.tensor_copy(ht[:], pst[:])

                    # ---- fc2: Y[t, d] = sum_h hT[h, t] * W2T[h, d]
                    psy = psyp.tile([CB, D], f32, tag="psy")
                    for ch in range(HC):
                        nc.tensor.matmul(
                            psy[:],
                            ht[:, ch * CB:(ch + 1) * CB],
                            w2[:, ch * D: (ch + 1) * D],
                            start=(ch == 0),
                            stop=(ch == HC - 1),
                        )

                    ybuf = ypool.tile([CB, D], f32, tag="y")
                    nc.vector.tensor_copy(ybuf[:], psy[:])
                    nc.scalar.dma_start(yt[s, b], ybuf[:])

    nc.compile()
    return nc


def _route(expert_idx):
    idx = np.asarray(expert_idx).astype(np.int64)
    order = np.argsort(idx, kind="stable")
    counts = np.bincount(idx, minlength=E)
    starts = np.zeros(E + 1, dtype=np.int64)
    starts[1:] = np.cumsum(counts)
    return order, starts, counts


def _pack_inputs(x, fc1_w, fc2_w, order, starts, C, np_dtype=np.float32):
    in_maps = []
    for core in range(NCORES):
        wh = np.empty((EPC, 128, WCOLS), np_dtype)
        xh = np.zeros((128, EPC * DC * C), np_dtype)
        for s in range(EPC):
            e = core * EPC + s
            # W1^T = fc1_w[e].T : [D, H]; d = c*128 + p -> col c*H + h
            w1t = np.ascontiguousarray(fc1_w[e].T).reshape(DC, 128, H)
            wh[s, :, :DC * H] = w1t.transpose(1, 0, 2).reshape(128, DC * H)
            # W2^T = fc2_w[e].T : [H, D]; h = ch*128 + p -> col DC*H + ch*D + d
            w2t = np.ascontiguousarray(fc2_w[e].T).reshape(HC, 128, D)
            wh[s, :, DC * H:] = w2t.transpose(1, 0, 2).reshape(128, HC * D)

            toks = order[starts[e]:starts[e + 1]]
            n = len(toks)
            if n:
                xte = np.ascontiguousarray(x[toks].T).reshape(DC, 128, n)
                for c in range(DC):
                    base = (s * DC + c) * C
                    xh[:, base:base + n] = xte[c]
        in_maps.append({"wts": wh, "xt": xh,
                        "idt": np.eye(CB, dtype=np_dtype)})
    return in_maps


def _unpack_outputs(results, order, starts, C, out_dtype):
    out = np.zeros((T, D), out_dtype)
    for core in range(NCORES):
        yh = np.asarray(results[core]["yt"]).reshape(EPC, C, D)
        for s in range(EPC):
            e = core * EPC + s
            toks = order[starts[e]:starts[e + 1]]
            n = len(toks)
            if n:
                out[toks] = yh[s, :n]
    return out


def kernel(x, expert_idx, fc1_w, fc2_w):
    from concourse.bass_utils import run_bass_kernel_spmd

    x = np.asarray(x, dtype=np.float32)
    fc1_w = np.asarray(fc1_w, dtype=np.float32)
    fc2_w = np.asarray(fc2_w, dtype=np.float32)

    order, starts, counts = _route(expert_idx)
    C = max(CB, int(-(-int(counts.max()) // CB) * CB))

    if C not in _prog_cache:
        _prog_cache[C] = _build_program(C)
    nc = _prog_cache[C]

    in_maps = _pack_inputs(x, fc1_w, fc2_w, order, starts, C)
    res = run_bass_kernel_spmd(nc, in_maps, list(range(NCORES)))
    return _unpack_outputs(res.results, order, starts, C, np.float32)

